# revision 8
# baseline (speedup 1.0000x reference)
"""2D Daubechies-2 DWT on Trainium2 — all-TensorE design, bf16, 8-core DP.

Input  x: [16, 1024, 1024, 1] f32  ->  Output: [16, 512, 512, 4] f32
Per core: 2 images. Host casts input to bf16 and builds tiny banded filter
matrices; device does both wavelet passes as PE matmuls:

  pass1 (column DWT, contract over h):  MT[w, i] = X_chunk.T @ W_t
    - X chunks [128 h x 128 w] are the STATIONARY operand so the output
      comes out w-major (transposed), which is exactly what pass 2 needs.
    - W is one banded [128 x 130] moving matrix reused for every h-tile
      (cols = 65-wide output window per filter); the t=0 variant bakes the
      symmetric top-mirror into its coefficients. Window overlap columns
      are handled with 1-col accumulate matmuls (start=False).
  drain: PSUM f32 -> SBUF bf16 copies, split ScalarE/VectorE.
  pass2 (row DWT, contract over w): YT[w', i] = R_pat.T @ MT_tile with
    banded stationary R patterns (A0/A/B/C per filter, mirrors baked).
  Output stored planar bf16 [img][c][w'][h']; host transposes to NHWC f32.
"""
import math

import numpy as np
import ml_dtypes

import concourse.bass as bass
import concourse.tile as tile
from concourse import bacc, mybir
from concourse.bass_utils import run_bass_kernel_spmd

N_CORES = 8
IMGS = 2
IMG_ELEMS = 1024 * 1024
OUT_ELEMS = 4 * 512 * 512
F32 = mybir.dt.float32
BF16 = mybir.dt.bfloat16
NPBF16 = ml_dtypes.bfloat16

_S3 = math.sqrt(3.0)
_DEN = 4.0 * math.sqrt(2.0)
H4 = np.array([(1 + _S3) / _DEN, (3 + _S3) / _DEN,
               (3 - _S3) / _DEN, (1 - _S3) / _DEN], dtype=np.float64)
G4 = np.array([H4[3], -H4[2], H4[1], -H4[0]], dtype=np.float64)

P_ORDER = {"A0": 0, "A": 1, "B": 2, "C": 3}


def _make_wmat():
    """[128, 260] f32: cols 0-129 = W0 (t=0, mirror baked), 130-259 = Wn."""
    W = np.zeros((128, 130), dtype=np.float64)
    for c in range(65):
        for k in range(4):
            r = 2 * c + k - 2
            if 0 <= r < 128:
                W[r, c] += H4[k]
                W[r, 65 + c] += G4[k]
    W0 = W.copy()
    W0[1, 0] += H4[0]
    W0[0, 0] += H4[1]
    W0[1, 65] += G4[0]
    W0[0, 65] += G4[1]
    return np.concatenate([W0, W], axis=1).astype(np.float32)


def _make_rmat():
    """[128, 1024] f32: col block (fr*4 + P_ORDER[pat])*128 = pattern."""
    out = np.zeros((128, 1024), dtype=np.float64)
    for fr, coeff in ((0, H4), (1, G4)):
        A = np.zeros((128, 128), dtype=np.float64)
        B = np.zeros((128, 128), dtype=np.float64)
        C = np.zeros((128, 128), dtype=np.float64)
        for c in range(128):
            for k in range(4):
                r = 2 * c + k - 2
                if 0 <= r < 128:
                    A[r, c] += coeff[k]
                rb = 2 * c + k - 130
                if 0 <= rb < 128:
                    B[rb, c] += coeff[k]
                rc = 2 * c + k + 126
                if 0 <= rc < 128:
                    C[rc, c] += coeff[k]
        A0 = A.copy()
        A0[1, 0] += coeff[0]
        A0[0, 0] += coeff[1]
        for name, m in (("A0", A0), ("A", A), ("B", B), ("C", C)):
            out[:, (fr * 4 + P_ORDER[name]) * 128:
                (fr * 4 + P_ORDER[name]) * 128 + 128] = m
    return out.astype(np.float32)


def _ap(handle, offset, dims):
    return bass.AP(handle, offset, [list(d) for d in dims])


def _tap(t, off, dims, pcnt=128, poff=0):
    f = t[:]
    pitch = f.ap[0][0]
    return bass.AP(f.tensor, f.offset + poff * pitch + off,
                   [[pitch, pcnt]] + [list(d) for d in dims])


def _build(reps=1, loop=False, dbg_mt=False):
    nc = bacc.Bacc("TRN2", target_bir_lowering=False, debug=False,
                   num_devices=1)
    xh = nc.dram_tensor("x", [IMGS * IMG_ELEMS], BF16, kind="ExternalInput")
    wh = nc.dram_tensor("wmat", [128 * 260], BF16, kind="ExternalInput")
    rh = nc.dram_tensor("rmat", [128 * 1024], BF16, kind="ExternalInput")
    yh = nc.dram_tensor("y", [IMGS * OUT_ELEMS], BF16, kind="ExternalOutput")

    with tile.TileContext(nc) as tc:
        with (
            tc.tile_pool(name="xs", bufs=2) as px,
            tc.tile_pool(name="mt", bufs=2) as pmt,
            tc.tile_pool(name="yb", bufs=2) as py,
            tc.tile_pool(name="cst", bufs=1) as pc,
            tc.tile_pool(name="pp1", bufs=4, space="PSUM") as pp1,
            tc.tile_pool(name="pp2", bufs=4, space="PSUM") as pp2,
        ):
            def body():
                Wt = pc.tile([128, 260], BF16, tag="wc")
                Rt = pc.tile([128, 1024], BF16, tag="rc")
                nc.sync.dma_start(Wt[:], _ap(wh, 0, [[260, 128], [1, 260]]))
                nc.sync.dma_start(Rt[:], _ap(rh, 0, [[1024, 128], [1, 1024]]))

                for img in range(IMGS):
                    X = px.tile([128, 8192], BF16, tag="xt")
                    nc.sync.dma_start(
                        _tap(X, 0, [[1024, 8], [1, 1024]]),
                        _ap(xh, img * IMG_ELEMS,
                            [[1024, 128], [131072, 8], [1, 1024]]))
                    MT = pmt.tile([128, 8192], BF16, tag="mtt")

                    # ---- pass 1: column DWT -> MT[w, i] ----
                    for c in range(8):
                        psL = pp1.tile([128, 512], F32, tag="p1")
                        psH = pp1.tile([128, 512], F32, tag="p1")
                        for t in range(8):
                            lhsT = _tap(X, t * 1024 + c * 128, [[1, 128]])
                            wofs = 0 if t == 0 else 130
                            n = 64 if t == 7 else 65
                            for f, ps in ((0, psL), (1, psH)):
                                nc.tensor.matmul(
                                    _tap(ps, 64 * t, [[1, n]]),
                                    lhsT,
                                    _tap(Wt, wofs + f * 65, [[1, n]]),
                                    start=(t == 0), stop=(t == 7),
                                    skip_group_check=True)
                        nc.scalar.activation(
                            _tap(MT, c * 1024, [[1, 512]]), psL[:],
                            mybir.ActivationFunctionType.Copy)
                        nc.vector.tensor_copy(
                            _tap(MT, c * 1024 + 512, [[1, 512]]), psH[:])

                    if dbg_mt:
                        nc.gpsimd.dma_start(
                            _ap(yh, img * OUT_ELEMS,
                                [[8192, 128], [1, 8192]]),
                            MT[:])
                        continue

                    # ---- pass 2: row DWT -> YT[w', i] ----
                    Y = py.tile([128, 8192], BF16, tag="yt")
                    for fr in range(2):
                        for m in range(4):
                            b = fr * 4 + m
                            if m == 0:
                                mml = [(0, "A0"), (1, "B")]
                            else:
                                mml = [(2 * m - 1, "C"), (2 * m, "A"),
                                       (2 * m + 1, "B")]
                            ps0 = pp2.tile([128, 512], F32, tag="p2")
                            ps1 = pp2.tile([128, 512], F32, tag="p2")
                            for idx, (t, pat) in enumerate(mml):
                                lhsT = _tap(
                                    Rt, (fr * 4 + P_ORDER[pat]) * 128,
                                    [[1, 128]])
                                st = idx == 0
                                sp = idx == len(mml) - 1
                                for half, ps in ((0, ps0), (1, ps1)):
                                    nc.tensor.matmul(
                                        ps[:], lhsT,
                                        _tap(MT, t * 1024 + half * 512,
                                             [[1, 512]]),
                                        start=st, stop=sp)
                            nc.scalar.activation(
                                _tap(Y, b * 1024, [[1, 512]]), ps0[:],
                                mybir.ActivationFunctionType.Copy)
                            nc.vector.tensor_copy(
                                _tap(Y, b * 1024 + 512, [[1, 512]]), ps1[:])

                            # store this block's two channel slabs as soon
                            # as they drain; alternate DMA queues so the
                            # descriptor-gen cost stays off one sequencer
                            for fc, dma in ((0, nc.gpsimd.dma_start),
                                            (1, nc.sync.dma_start)):
                                ch = 2 * fr + fc
                                dma(
                                    _ap(yh, img * OUT_ELEMS + ch * 262144
                                        + m * 65536,
                                        [[512, 128], [1, 512]]),
                                    _tap(Y, b * 1024 + fc * 512,
                                         [[1, 512]]))

            if loop and reps > 1:
                with tc.For_i(0, reps, 1):
                    body()
            else:
                for _rep in range(reps):
                    body()
    nc.compile()
    return nc


_NC_CACHE = {}


def _get_nc(reps=1, loop=False):
    key = (reps, loop)
    if key not in _NC_CACHE:
        _NC_CACHE[key] = _build(reps, loop)
    return _NC_CACHE[key]


def _const_maps():
    w = _make_wmat().astype(NPBF16).ravel()
    r = _make_rmat().astype(NPBF16).ravel()
    return w, r


def kernel(**inputs):
    x = np.asarray(inputs["x"], dtype=np.float32)
    assert x.shape == (16, 1024, 1024, 1), x.shape
    nc = _get_nc(1)
    xb = x.reshape(N_CORES, IMGS * IMG_ELEMS).astype(NPBF16)
    w, r = _const_maps()
    in_maps = [{"x": xb[i], "wmat": w, "rmat": r} for i in range(N_CORES)]
    res = run_bass_kernel_spmd(nc, in_maps, core_ids=list(range(N_CORES)))
    # y planar [img][c][w'][h'] bf16 -> [16, h', w', c] f32
    full = np.stack([np.asarray(res.results[i]["y"]).reshape(
        IMGS, 4, 512, 512) for i in range(N_CORES)])
    out = full.transpose(0, 1, 4, 3, 2).reshape(16, 512, 512, 4)
    return np.ascontiguousarray(out).astype(np.float32)


# revision 10
# speedup vs baseline: 1.9558x; 1.9558x over previous
"""2D Daubechies-2 DWT on Trainium2 — all-TensorE design, bf16, 8-core DP.

Input  x: [16, 1024, 1024, 1] f32  ->  Output: [16, 512, 512, 4] f32
Per core: 2 images. Host casts input to bf16 and builds tiny banded filter
matrices; device does both wavelet passes as PE matmuls:

  pass1 (column DWT, contract over h):  MT[w, i] = X_chunk.T @ W_t
    - X chunks [128 h x 128 w] are the STATIONARY operand so the output
      comes out w-major (transposed), which is exactly what pass 2 needs.
    - W is one banded [128 x 130] moving matrix reused for every h-tile
      (cols = 65-wide output window per filter); the t=0 variant bakes the
      symmetric top-mirror into its coefficients. Window overlap columns
      are handled with 1-col accumulate matmuls (start=False).
  drain: PSUM f32 -> SBUF bf16 copies, split ScalarE/VectorE.
  pass2 (row DWT, contract over w): YT[w', i] = R_pat.T @ MT_tile with
    banded stationary R patterns (A0/A/B/C per filter, mirrors baked).
  Output stored planar bf16 [img][c][w'][h']; host transposes to NHWC f32.
"""
import math

import numpy as np
import ml_dtypes

import concourse.bass as bass
import concourse.tile as tile
from concourse import bacc, mybir
from concourse.bass_utils import run_bass_kernel_spmd

N_CORES = 8
IMGS = 2
IMG_ELEMS = 1024 * 1024
OUT_ELEMS = 4 * 512 * 512
F32 = mybir.dt.float32
BF16 = mybir.dt.bfloat16
NPBF16 = ml_dtypes.bfloat16

_S3 = math.sqrt(3.0)
_DEN = 4.0 * math.sqrt(2.0)
H4 = np.array([(1 + _S3) / _DEN, (3 + _S3) / _DEN,
               (3 - _S3) / _DEN, (1 - _S3) / _DEN], dtype=np.float64)
G4 = np.array([H4[3], -H4[2], H4[1], -H4[0]], dtype=np.float64)

P_ORDER = {"A0": 0, "A": 1, "B": 2, "C": 3}


def _make_wmat():
    """[128, 260] f32: cols 0-129 = W0 (t=0, mirror baked), 130-259 = Wn."""
    W = np.zeros((128, 130), dtype=np.float64)
    for c in range(65):
        for k in range(4):
            r = 2 * c + k - 2
            if 0 <= r < 128:
                W[r, c] += H4[k]
                W[r, 65 + c] += G4[k]
    W0 = W.copy()
    W0[1, 0] += H4[0]
    W0[0, 0] += H4[1]
    W0[1, 65] += G4[0]
    W0[0, 65] += G4[1]
    return np.concatenate([W0, W], axis=1).astype(np.float32)


def _make_rmat():
    """[128, 1024] f32: col block (fr*4 + P_ORDER[pat])*128 = pattern."""
    out = np.zeros((128, 1024), dtype=np.float64)
    for fr, coeff in ((0, H4), (1, G4)):
        A = np.zeros((128, 128), dtype=np.float64)
        B = np.zeros((128, 128), dtype=np.float64)
        C = np.zeros((128, 128), dtype=np.float64)
        for c in range(128):
            for k in range(4):
                r = 2 * c + k - 2
                if 0 <= r < 128:
                    A[r, c] += coeff[k]
                rb = 2 * c + k - 130
                if 0 <= rb < 128:
                    B[rb, c] += coeff[k]
                rc = 2 * c + k + 126
                if 0 <= rc < 128:
                    C[rc, c] += coeff[k]
        A0 = A.copy()
        A0[1, 0] += coeff[0]
        A0[0, 0] += coeff[1]
        for name, m in (("A0", A0), ("A", A), ("B", B), ("C", C)):
            out[:, (fr * 4 + P_ORDER[name]) * 128:
                (fr * 4 + P_ORDER[name]) * 128 + 128] = m
    return out.astype(np.float32)


def _ap(handle, offset, dims):
    return bass.AP(handle, offset, [list(d) for d in dims])


def _tap(t, off, dims, pcnt=128, poff=0):
    f = t[:]
    pitch = f.ap[0][0]
    return bass.AP(f.tensor, f.offset + poff * pitch + off,
                   [[pitch, pcnt]] + [list(d) for d in dims])


def _build(reps=1, loop=False, dbg_mt=False):
    nc = bacc.Bacc("TRN2", target_bir_lowering=False, debug=False,
                   num_devices=1)
    xh = nc.dram_tensor("x", [IMGS * IMG_ELEMS], BF16, kind="ExternalInput")
    wh = nc.dram_tensor("wmat", [128 * 260], BF16, kind="ExternalInput")
    rh = nc.dram_tensor("rmat", [128 * 1024], BF16, kind="ExternalInput")
    yh = nc.dram_tensor("y", [IMGS * OUT_ELEMS], BF16, kind="ExternalOutput")

    with tile.TileContext(nc) as tc:
        with (
            tc.tile_pool(name="xs", bufs=2) as px,
            tc.tile_pool(name="mt", bufs=2) as pmt,
            tc.tile_pool(name="yb", bufs=2) as py,
            tc.tile_pool(name="cst", bufs=1) as pc,
            tc.tile_pool(name="pp1", bufs=4, space="PSUM") as pp1,
            tc.tile_pool(name="pp2", bufs=4, space="PSUM") as pp2,
        ):
            def body():
                Wt = pc.tile([128, 260], BF16, tag="wc")
                Rt = pc.tile([128, 1024], BF16, tag="rc")

                for img in range(IMGS):
                    # load X in w-halves: chunks 0-3 only need cols 0-511,
                    # so pass 1 starts after half the load; the tiny const
                    # DMAs ride between the halves instead of ahead of them
                    X = px.tile([128, 8192], BF16, tag="xt")
                    for half in range(2):
                        nc.sync.dma_start(
                            _tap(X, half * 512, [[1024, 8], [1, 512]]),
                            _ap(xh, img * IMG_ELEMS + half * 512,
                                [[1024, 128], [131072, 8], [1, 512]]))
                        if img == 0 and half == 0:
                            nc.sync.dma_start(
                                Wt[:], _ap(wh, 0, [[260, 128], [1, 260]]))
                            nc.sync.dma_start(
                                Rt[:], _ap(rh, 0, [[1024, 128], [1, 1024]]))
                    MT = pmt.tile([128, 8192], BF16, tag="mtt")

                    # ---- pass 1: column DWT -> MT[w, i] ----
                    for c in range(8):
                        psL = pp1.tile([128, 512], F32, tag="p1")
                        psH = pp1.tile([128, 512], F32, tag="p1")
                        for t in range(8):
                            lhsT = _tap(X, t * 1024 + c * 128, [[1, 128]])
                            wofs = 0 if t == 0 else 130
                            n = 64 if t == 7 else 65
                            for f, ps in ((0, psL), (1, psH)):
                                nc.tensor.matmul(
                                    _tap(ps, 64 * t, [[1, n]]),
                                    lhsT,
                                    _tap(Wt, wofs + f * 65, [[1, n]]),
                                    start=(t == 0), stop=(t == 7),
                                    skip_group_check=True)
                        nc.scalar.activation(
                            _tap(MT, c * 1024, [[1, 512]]), psL[:],
                            mybir.ActivationFunctionType.Copy)
                        nc.vector.tensor_copy(
                            _tap(MT, c * 1024 + 512, [[1, 512]]), psH[:])

                    if dbg_mt:
                        nc.gpsimd.dma_start(
                            _ap(yh, img * OUT_ELEMS,
                                [[8192, 128], [1, 8192]]),
                            MT[:])
                        continue

                    # ---- pass 2: row DWT -> YT[w', i] ----
                    Y = py.tile([128, 8192], BF16, tag="yt")
                    for fr in range(2):
                        for m in range(4):
                            b = fr * 4 + m
                            if m == 0:
                                mml = [(0, "A0"), (1, "B")]
                            else:
                                mml = [(2 * m - 1, "C"), (2 * m, "A"),
                                       (2 * m + 1, "B")]
                            ps0 = pp2.tile([128, 512], F32, tag="p2")
                            ps1 = pp2.tile([128, 512], F32, tag="p2")
                            for idx, (t, pat) in enumerate(mml):
                                lhsT = _tap(
                                    Rt, (fr * 4 + P_ORDER[pat]) * 128,
                                    [[1, 128]])
                                st = idx == 0
                                sp = idx == len(mml) - 1
                                for half, ps in ((0, ps0), (1, ps1)):
                                    nc.tensor.matmul(
                                        ps[:], lhsT,
                                        _tap(MT, t * 1024 + half * 512,
                                             [[1, 512]]),
                                        start=st, stop=sp)
                            nc.scalar.activation(
                                _tap(Y, b * 1024, [[1, 512]]), ps0[:],
                                mybir.ActivationFunctionType.Copy)
                            nc.vector.tensor_copy(
                                _tap(Y, b * 1024 + 512, [[1, 512]]), ps1[:])

                        # store planar [c][w'][h'] as soon as this row
                        # filter's 4 blocks are drained (one DMA/channel)
                        for fc in range(2):
                            ch = 2 * fr + fc
                            nc.gpsimd.dma_start(
                                _ap(yh, img * OUT_ELEMS + ch * 262144,
                                    [[512, 128], [65536, 4], [1, 512]]),
                                _tap(Y, fr * 4096 + fc * 512,
                                     [[1024, 4], [1, 512]]))

            if loop and reps > 1:
                with tc.For_i(0, reps, 1):
                    body()
            else:
                for _rep in range(reps):
                    body()
    nc.compile()
    return nc


_NC_CACHE = {}


def _get_nc(reps=1, loop=False):
    key = (reps, loop)
    if key not in _NC_CACHE:
        _NC_CACHE[key] = _build(reps, loop)
    return _NC_CACHE[key]


def _const_maps():
    w = _make_wmat().astype(NPBF16).ravel()
    r = _make_rmat().astype(NPBF16).ravel()
    return w, r


def kernel(**inputs):
    x = np.asarray(inputs["x"], dtype=np.float32)
    assert x.shape == (16, 1024, 1024, 1), x.shape
    nc = _get_nc(1)
    xb = x.reshape(N_CORES, IMGS * IMG_ELEMS).astype(NPBF16)
    w, r = _const_maps()
    in_maps = [{"x": xb[i], "wmat": w, "rmat": r} for i in range(N_CORES)]
    res = run_bass_kernel_spmd(nc, in_maps, core_ids=list(range(N_CORES)))
    # y planar [img][c][w'][h'] bf16 -> [16, h', w', c] f32
    full = np.stack([np.asarray(res.results[i]["y"]).reshape(
        IMGS, 4, 512, 512) for i in range(N_CORES)])
    out = full.transpose(0, 1, 4, 3, 2).reshape(16, 512, 512, 4)
    return np.ascontiguousarray(out).astype(np.float32)
